# revision 20
# baseline (speedup 1.0000x reference)
"""Transformer block (dense_transformer) on 8 Trainium2 NeuronCores.

Sharding: pure data-parallel — batch B=8, one batch element per core; zero
collectives. Each core runs the full block on its [1024, 768] slice.

On-device layout: the residual stream is kept FEATURE-major ([768, 1024]:
features on SBUF partitions in 6 chunks of 128, tokens on the free axis).
The host transposes x once on the way in and the output once on the way out.

Precision: matmuls in float32r (e8m11, rounded on read, full PE rate at
free-dim >= 256); attention-internal tensors (exp(S), V, attn_out) in bf16;
PSUM accumulation fp32.  LayerNorm affine (w, b) is folded into the
downstream weights on the host.

Attention: scores are computed directly TRANSPOSED (S^T[tk,tq] = K Q^T per
head), exp'd without max-subtraction (scores are O(1); exp is safe in fp32),
and the AV matmul uses V augmented with a ones-column so the softmax
denominator Z falls out as row 64 of the output.  1/Z is applied before proj
via a GpSimd partition broadcast.
"""
import numpy as np
from contextlib import ExitStack

import concourse.bass as bass
import concourse.tile as tile
from concourse import bacc, mybir
from concourse.bass_utils import run_bass_kernel_spmd

f32 = mybir.dt.float32
f32r = mybir.dt.float32r
bf16 = mybir.dt.bfloat16
AF = mybir.ActivationFunctionType
OP = mybir.AluOpType

NCORES = 8
NT = 1024          # tokens per core (batch element)
DIM = 768
CD = DIM // 128    # 6 feature chunks
NH = 12
HD = 64
HID = 3072
HM = HID // 128    # 24 hidden chunks
EPS = 1e-5
HALVES = (slice(0, 512), slice(512, 1024))

_CACHE = {}


def _build(debug=False):
    nc = bacc.Bacc("TRN2", target_bir_lowering=False, debug=False,
                   enable_asserts=False, num_devices=NCORES)
    d = {}
    d["x_fm"] = nc.dram_tensor("x_fm", [DIM, NT], f32r, kind="ExternalInput").ap()
    d["qkv_wT"] = nc.dram_tensor("qkv_wT", [DIM, 3 * DIM], f32r, kind="ExternalInput").ap()
    d["proj_wTb"] = nc.dram_tensor("proj_wTb", [DIM, DIM], bf16, kind="ExternalInput").ap()
    d["fc1_wT"] = nc.dram_tensor("fc1_wT", [DIM, HID], f32r, kind="ExternalInput").ap()
    d["fc2_wT"] = nc.dram_tensor("fc2_wT", [HID, DIM], f32r, kind="ExternalInput").ap()
    d["qkvb"] = nc.dram_tensor("qkvb", [3 * DIM], f32, kind="ExternalInput").ap()
    d["pbg1"] = nc.dram_tensor("pbg1", [DIM], f32, kind="ExternalInput").ap()
    d["g1"] = nc.dram_tensor("g1", [DIM], f32, kind="ExternalInput").ap()
    d["fb1"] = nc.dram_tensor("fb1", [HID], f32, kind="ExternalInput").ap()
    d["fb2g2"] = nc.dram_tensor("fb2g2", [DIM], f32, kind="ExternalInput").ap()
    d["g2"] = nc.dram_tensor("g2", [DIM], f32, kind="ExternalInput").ap()
    out_fm = nc.dram_tensor("out_fm", [DIM, NT], f32, kind="ExternalOutput").ap()
    dbg = {}
    if debug:
        dbg["xn"] = nc.dram_tensor("dbg_xn", [CD, 128, NT], f32, kind="ExternalOutput").ap()
        dbg["mub"] = nc.dram_tensor("dbg_mub", [128, NT], f32, kind="ExternalOutput").ap()
        dbg["rstdb"] = nc.dram_tensor("dbg_rstdb", [128, NT], f32, kind="ExternalOutput").ap()
        dbg["q"] = nc.dram_tensor("dbg_q", [CD, 128, NT], f32, kind="ExternalOutput").ap()
        dbg["k"] = nc.dram_tensor("dbg_k", [CD, 128, NT], f32, kind="ExternalOutput").ap()
        dbg["v"] = nc.dram_tensor("dbg_v", [128, 8, NH, HD + 1], bf16, kind="ExternalOutput").ap()
        dbg["e0"] = nc.dram_tensor("dbg_e0", [8, 128, NT], bf16, kind="ExternalOutput").ap()
        dbg["z"] = nc.dram_tensor("dbg_z", [NH, NT], f32, kind="ExternalOutput").ap()
        dbg["rb"] = nc.dram_tensor("dbg_rb", [64, NT], f32, kind="ExternalOutput").ap()
        dbg["ao"] = nc.dram_tensor("dbg_ao", [CD, 128, NT], bf16, kind="ExternalOutput").ap()
        dbg["y1"] = nc.dram_tensor("dbg_y1", [CD, 128, NT], f32, kind="ExternalOutput").ap()
        dbg["h0"] = nc.dram_tensor("dbg_h0", [128, 512], f32, kind="ExternalOutput").ap()

    with tile.TileContext(nc) as tc, ExitStack() as ctx:
        sb = ctx.enter_context(tc.tile_pool(name="sb", bufs=1))

        # ---- constants ----
        ones_f = sb.tile([128, NH, 1], f32, tag="ones_f")
        nc.vector.memset(ones_f, 1.0)
        ones_r = sb.tile([128, 1], f32r, tag="ones_r")
        nc.vector.tensor_copy(out=ones_r, in_=ones_f[:, 0, :])
        ones_b = sb.tile([128, 1], bf16, tag="ones_b")
        nc.vector.tensor_copy(out=ones_b, in_=ones_f[:, 0, :])
        eps_t = sb.tile([128, 1], f32, tag="eps")
        nc.vector.memset(eps_t, EPS)
        qkvb_t = sb.tile([128, 18], f32, tag="qkvb")
        nc.sync.dma_start(out=qkvb_t, in_=d["qkvb"].rearrange("(c p) -> p c", p=128))
        g1_t = sb.tile([128, CD], f32, tag="g1")
        nc.sync.dma_start(out=g1_t, in_=d["g1"].rearrange("(c p) -> p c", p=128))
        pbg1_t = sb.tile([128, CD], f32, tag="pbg1")
        nc.sync.dma_start(out=pbg1_t, in_=d["pbg1"].rearrange("(c p) -> p c", p=128))
        g2_t = sb.tile([128, CD], f32, tag="g2")
        nc.sync.dma_start(out=g2_t, in_=d["g2"].rearrange("(c p) -> p c", p=128))
        fb2g2_t = sb.tile([128, CD], f32, tag="fb2g2")
        nc.sync.dma_start(out=fb2g2_t, in_=d["fb2g2"].rearrange("(c p) -> p c", p=128))
        fb1_t = sb.tile([128, HM], f32, tag="fb1")
        nc.sync.dma_start(out=fb1_t, in_=d["fb1"].rearrange("(c p) -> p c", p=128))
        vb_row = sb.tile([1, DIM], f32, tag="vb_row")
        nc.sync.dma_start(out=vb_row, in_=d["qkvb"][2 * DIM:3 * DIM])
        vb_b = sb.tile([128, DIM], f32, tag="vb_b")
        nc.gpsimd.partition_broadcast(vb_b, vb_row)

        # ---- persistent / streamed activations ----
        x_t = sb.tile([128, CD, NT], f32r, tag="x")          # slot reused for out_sb
        xn_t = sb.tile([128, CD, NT], f32r, tag="xn")        # slot reused for y1n
        v_aug = sb.tile([128, 8, NH, HD + 1], bf16, tag="vaug")
        ao_t = sb.tile([128, CD, NT], bf16, tag="ao")        # attn_out, feature-major
        y1_t = sb.tile([128, CD, NT], f32r, tag="y1")

        for kc in range(CD):
            nc.sync.dma_start(out=x_t[:, kc, :], in_=d["x_fm"][kc * 128:(kc + 1) * 128, :])

        # ================= helpers =================
        # weight loads rotate across DMA queues so no single engine queue
        # serializes the stream
        _dmaq = [nc.sync, nc.gpsimd, nc.scalar]
        _dmaq_i = [0]

        def wdma(out, in_):
            q = _dmaq[_dmaq_i[0] % len(_dmaq)]
            _dmaq_i[0] += 1
            q.dma_start(out=out, in_=in_)

        qkvw_r = d["qkv_wT"].rearrange("(c p) m -> p c m", p=128)
        projw_r = d["proj_wTb"].rearrange("(c p) m -> p c m", p=128)
        fc1w_r = d["fc1_wT"].rearrange("(c p) m -> p c m", p=128)

        def ln_feature_major(sfx, src, dst):
            """dst = (src - mean)/sqrt(var+eps); stats over features (partitions).
            src f32r [128, CD, NT] (read via bitcast for fp32 math); dst f32r."""
            src_f = src.bitcast(f32)
            xsq = {}
            for kc in range(CD):
                for hi, sl in enumerate(HALVES):
                    xq = sb.tile([128, 512], bf16, tag="e", bufs=16,
                                 name=f"xsq{sfx}_{kc}_{hi}")
                    nc.vector.tensor_mul(xq, src_f[:, kc, sl], src_f[:, kc, sl])
                    xsq[(kc, hi)] = xq
            with tc.tile_pool(name=f"ps_ln{sfx}", bufs=1, space="PSUM") as ps:
                s1 = ps.tile([1, NT], f32, tag="s1")
                s2 = ps.tile([1, NT], f32, tag="s2")
                for hi, sl in enumerate(HALVES):
                    for kc in range(CD):
                        nc.tensor.matmul(s1[0:1, sl], ones_r, src[:, kc, sl],
                                         start=(kc == 0), stop=(kc == CD - 1))
                    for kc in range(CD):
                        nc.tensor.matmul(s2[0:1, sl], ones_b, xsq[(kc, hi)],
                                         start=(kc == 0), stop=(kc == CD - 1))
                mu = sb.tile([1, NT], f32, tag="stt", bufs=3, name=f"mu{sfx}")
                nc.vector.tensor_scalar_mul(mu, s1[0:1, :], 1.0 / DIM)
                ex2 = sb.tile([1, NT], f32, tag="stt", bufs=3, name=f"ex2{sfx}")
                nc.vector.tensor_scalar_mul(ex2, s2[0:1, :], 1.0 / DIM)
            mu_b = sb.tile([128, NT], f32, tag="bc", bufs=2, name=f"mub{sfx}")
            nc.gpsimd.partition_broadcast(mu_b, mu)
            # rstd math in a [128, 8] layout — single-partition [1,1024] DVE
            # ops cost ~6.5us each; reshaped they are ~0.1us.
            mu_p = sb.tile([128, 8], f32, tag="stp", bufs=4, name=f"mup{sfx}")
            nc.scalar.dma_start(out=mu_p, in_=mu)
            ex2_p = sb.tile([128, 8], f32, tag="stp", bufs=4, name=f"ex2p{sfx}")
            nc.scalar.dma_start(out=ex2_p, in_=ex2)
            nc.vector.tensor_mul(mu_p, mu_p, mu_p)          # mu^2
            nc.vector.tensor_sub(ex2_p, ex2_p, mu_p)        # var
            nc.scalar.activation(out=ex2_p, in_=ex2_p, func=AF.Sqrt, bias=eps_t)
            nc.vector.reciprocal(ex2_p, ex2_p)              # rstd
            rstd_row = sb.tile([1, NT], f32, tag="stt", bufs=3, name=f"rsr{sfx}")
            nc.scalar.dma_start(out=rstd_row, in_=ex2_p)
            rstd_b = sb.tile([128, NT], f32, tag="bc", bufs=2, name=f"rstdb{sfx}")
            nc.gpsimd.partition_broadcast(rstd_b, rstd_row)
            if debug and sfx == "1":
                nc.gpsimd.dma_start(out=dbg["mub"], in_=mu_b)
                nc.gpsimd.dma_start(out=dbg["rstdb"], in_=rstd_b)
            for kc in range(CD):
                t = sb.tile([128, NT], f32, tag="scr", bufs=2, name=f"lnt{sfx}_{kc}")
                nc.vector.tensor_sub(t, src_f[:, kc, :], mu_b)
                nc.vector.tensor_mul(dst[:, kc, :], t, rstd_b)

        # ================= LN1 =================
        ln_feature_major("1", x_t, xn_t)
        if debug:
            for kc in range(CD):
                nc.gpsimd.dma_start(out=dbg["xn"][kc], in_=xn_t[:, kc, :].bitcast(f32))

        # ====== QKV + attention (shared PSUM pool: 2+2+4 = 8 banks) ======
        q_tiles = {}
        k_tiles = {}
        with tc.tile_pool(name="ps_qa", bufs=1, space="PSUM") as ps:
            # --- v FIRST (attention depends on it; emitting it after q/k
            # deadlocks the mm-tag slot cycle against the qk-tile cycle) ---
            for t in range(8):
                nc.vector.tensor_copy(out=v_aug[:, t, :, HD:HD + 1], in_=ones_f)
            wv_tiles = []
            for kc in range(CD):
                wv = sb.tile([128, DIM], f32r, tag="w768", bufs=6, name=f"wv{kc}")
                nc.sync.dma_start(out=wv, in_=d["qkv_wT"][kc * 128:(kc + 1) * 128,
                                                          2 * DIM:3 * DIM])
                wv_tiles.append(wv)
            for t in range(8):
                for j in range(2):
                    vsl = slice(j * 384, (j + 1) * 384)
                    acc = ps.tile([128, 384], f32, tag="mm", bufs=2, name=f"v_ps{t}_{j}")
                    for kc in range(CD):
                        nc.tensor.matmul(acc,
                                         xn_t[:, kc, t * 128:(t + 1) * 128],
                                         wv_tiles[kc][:, vsl],
                                         start=(kc == 0), stop=(kc == CD - 1))
                    nc.vector.tensor_add(
                        v_aug[:, t, 6 * j:6 * (j + 1), 0:HD],
                        acc.rearrange("p (h d) -> p h d", h=6),
                        vb_b[:, vsl].rearrange("p (h d) -> p h d", h=6))

            if debug:
                nc.gpsimd.dma_start(out=dbg["v"], in_=v_aug)

            # --- q (m 0..5) / k (m 6..11), interleaved per pair ---
            for p in range(CD):
                for m in (p, p + CD):
                    w6 = sb.tile([128, CD, 128], f32r, tag="w6", bufs=4,
                                 name=f"wqk{m}")
                    wdma(w6, qkvw_r[:, :, m * 128:(m + 1) * 128])
                    dst = sb.tile([128, NT], f32r, tag="qk", bufs=4,
                                  name=f"{'q' if m < CD else 'k'}{p}")
                    for sl in HALVES:
                        acc = ps.tile([128, 512], f32, tag="mm", bufs=2, name=f"qk_ps{m}")
                        for kc in range(CD):
                            nc.tensor.matmul(acc, w6[:, kc, :], xn_t[:, kc, sl],
                                             start=(kc == 0), stop=(kc == CD - 1))
                        if m < CD:  # q: (psum + qb) * head_dim^-0.5
                            nc.vector.tensor_scalar(out=dst[:, sl], in0=acc,
                                                    scalar1=qkvb_t[:, m:m + 1],
                                                    scalar2=float(HD) ** -0.5,
                                                    op0=OP.add, op1=OP.mult)
                        else:       # k: psum + kb
                            nc.scalar.activation(out=dst[:, sl], in_=acc,
                                                 func=AF.Identity,
                                                 bias=qkvb_t[:, m:m + 1])
                    if m < CD:
                        q_tiles[p] = dst
                        if debug:
                            nc.gpsimd.dma_start(out=dbg["q"][p], in_=dst.bitcast(f32))
                    else:
                        k_tiles[p] = dst
                        if debug:
                            nc.gpsimd.dma_start(out=dbg["k"][p], in_=dst.bitcast(f32))

            # --- attention, head pair p -> heads (2p, 2p+1) ---
            for p in range(CD):
                avps = [ps.tile([128, NT], f32, tag="av", bufs=2, name=f"av{p}_{h2}")
                        for h2 in range(2)]
                for s in range(8):
                    for hi, sl in enumerate(HALVES):
                        # the two heads' S^T matmuls issue back-to-back into
                        # different row groups -> they run concurrently on PE
                        stps = []
                        for h2 in range(2):
                            base = h2 * 64
                            stp = ps.tile([128, 512], f32, tag="st", bufs=2,
                                          name=f"st{p}_{s}_{h2}_{hi}")
                            nc.tensor.matmul(stp,
                                             k_tiles[p][base:base + 64,
                                                        s * 128:(s + 1) * 128],
                                             q_tiles[p][base:base + 64, sl],
                                             start=True, stop=True,
                                             tile_position=(base, 0))
                            stps.append(stp)
                        for h2 in range(2):
                            eT = sb.tile([128, 512], bf16, tag="e", bufs=16,
                                         name=f"e{p}_{s}_{h2}_{hi}")
                            nc.scalar.activation(out=eT, in_=stps[h2], func=AF.Exp)
                            if debug and p == 0 and h2 == 0:
                                nc.gpsimd.dma_start(out=dbg["e0"][s, :, sl], in_=eT)
                            nc.tensor.matmul(avps[h2][0:HD + 1, sl],
                                             v_aug[:, s, 2 * p + h2, :],
                                             eT,
                                             start=(s == 0), stop=(s == 7))
                for h2 in range(2):
                    # Z row lives at PSUM partition 64.  Copy it out (ACT),
                    # reshape to [128, 8] via DMA so the reciprocal runs on
                    # 128 lanes, reshape back to partition 0, broadcast.
                    zs = sb.tile([65, NT], f32, tag="scr", bufs=2, name=f"zs{p}_{h2}")
                    nc.scalar.copy(out=zs[HD:HD + 1, :], in_=avps[h2][HD:HD + 1, :])
                    zp = sb.tile([128, 8], f32, tag="stp", bufs=4, name=f"zp{p}_{h2}")
                    nc.scalar.dma_start(out=zp, in_=zs[HD:HD + 1, :])
                    nc.vector.reciprocal(zp, zp)
                    zr = sb.tile([1, NT], f32, tag="stt", bufs=3, name=f"zr{p}_{h2}")
                    nc.scalar.dma_start(out=zr, in_=zp)
                    if debug:
                        nc.gpsimd.dma_start(out=dbg["z"][2 * p + h2], in_=zr)
                    r_b = sb.tile([64, NT], f32, tag="bc", bufs=2, name=f"rb{p}_{h2}")
                    nc.gpsimd.partition_broadcast(r_b, zr)
                    if debug and p == 0 and h2 == 0:
                        nc.gpsimd.dma_start(out=dbg["rb"], in_=r_b)
                    if h2 == 0:
                        nc.vector.tensor_mul(ao_t[0:64, p, :], avps[0][0:64, :], r_b)
                    else:
                        tmp = sb.tile([64, NT], bf16, tag="h", bufs=2, name=f"sh{p}")
                        nc.vector.tensor_mul(tmp, avps[1][0:64, :], r_b)
                        nc.gpsimd.dma_start(out=ao_t[64:128, p, :], in_=tmp)

        # ================= proj + residual =================
        with tc.tile_pool(name="ps_proj", bufs=1, space="PSUM") as ps:
            for m in range(CD):
                w6 = sb.tile([128, CD, 128], bf16, tag="w6", bufs=4, name=f"wpj{m}")
                wdma(w6, projw_r[:, :, m * 128:(m + 1) * 128])
                for sl in HALVES:
                    acc = ps.tile([128, 512], f32, tag="mm", bufs=2, name=f"pj_ps{m}")
                    for cc in range(CD):
                        nc.tensor.matmul(acc, w6[:, cc, :], ao_t[:, cc, sl],
                                         start=(cc == 0), stop=(cc == CD - 1))
                    tmp = sb.tile([128, 512], f32, tag="scr512", bufs=2, name=f"pjt{m}")
                    nc.scalar.activation(out=tmp, in_=acc, func=AF.Identity,
                                         bias=pbg1_t[:, m:m + 1], scale=g1_t[:, m:m + 1])
                    nc.vector.tensor_add(y1_t[:, m, sl], tmp, x_t[:, m, sl].bitcast(f32))

        if debug:
            for kc in range(CD):
                nc.gpsimd.dma_start(out=dbg["ao"][kc], in_=ao_t[:, kc, :])
                nc.gpsimd.dma_start(out=dbg["y1"][kc], in_=y1_t[:, kc, :].bitcast(f32))

        # ================= LN2 =================
        y1n_t = sb.tile([128, CD, NT], f32r, tag="xn", name="y1n")
        ln_feature_major("2", y1_t, y1n_t)

        # ================= MLP =================
        out_sb = sb.tile([128, CD, NT], f32, tag="x", name="out_sb")
        with tc.tile_pool(name="ps_mlp", bufs=1, space="PSUM") as ps:
            for hi, sl in enumerate(HALVES):
                acc2 = ps.tile([128, CD, 512], f32, tag="fc2", bufs=1, name=f"fc2_ps{hi}")
                for hm in range(HM):
                    fps = ps.tile([128, 512], f32, tag="fc1", bufs=2,
                                  name=f"fc1_ps{hi}_{hm}")
                    w6 = sb.tile([128, CD, 128], f32r, tag="w6", bufs=4,
                                 name=f"wf1_{hi}_{hm}")
                    wdma(w6, fc1w_r[:, :, hm * 128:(hm + 1) * 128])
                    for cc in range(CD):
                        nc.tensor.matmul(fps, w6[:, cc, :], y1n_t[:, cc, sl],
                                         start=(cc == 0), stop=(cc == CD - 1))
                    h_t = sb.tile([128, 512], f32r, tag="h", bufs=2, name=f"h{hi}_{hm}")
                    nc.scalar.activation(out=h_t, in_=fps, func=AF.Gelu,
                                         bias=fb1_t[:, hm:hm + 1])
                    if debug and hi == 0 and hm == 0:
                        nc.gpsimd.dma_start(out=dbg["h0"], in_=h_t.bitcast(f32))
                    w2 = sb.tile([128, DIM], f32r, tag="w768", bufs=6,
                                 name=f"wf2_{hi}_{hm}")
                    wdma(w2, d["fc2_wT"][hm * 128:(hm + 1) * 128, :])
                    for m in range(CD):
                        nc.tensor.matmul(acc2[:, m, :], w2[:, m * 128:(m + 1) * 128], h_t,
                                         start=(hm == 0), stop=(hm == HM - 1))
                for m in range(CD):
                    tmp = sb.tile([128, 512], f32, tag="scr512", bufs=2,
                                  name=f"of{hi}_{m}")
                    nc.scalar.activation(out=tmp, in_=acc2[:, m, :], func=AF.Identity,
                                         bias=fb2g2_t[:, m:m + 1], scale=g2_t[:, m:m + 1])
                    nc.vector.tensor_add(out_sb[:, m, sl], tmp,
                                         y1_t[:, m, sl].bitcast(f32))

        for kc in range(CD):
            nc.gpsimd.dma_start(out=out_fm[kc * 128:(kc + 1) * 128, :],
                                in_=out_sb[:, kc, :])

    nc.compile()
    return nc


def kernel(x, ln1_w, ln1_b, qkv_w, q_bias, v_bias, proj_w, proj_b,
           ln2_w, ln2_b, fc1_w, fc1_b, fc2_w, fc2_b, gamma1, gamma2):
    import ml_dtypes
    if "nc" not in _CACHE:
        _CACHE["nc"] = _build()
    nc = _CACHE["nc"]

    f64 = np.float64
    x = np.asarray(x, np.float32)
    # Fold LN affine params into downstream weights (exact, on host):
    #   qkv(ln1(x)) = (qkv_w * ln1_w) @ xhat + (qkv_w @ ln1_b + qkv_bias)
    qkv_w64 = np.asarray(qkv_w, f64)
    qkv_bias = np.concatenate([np.asarray(q_bias, f64),
                               np.zeros(DIM, f64),
                               np.asarray(v_bias, f64)])
    qkv_w_f = qkv_w64 * np.asarray(ln1_w, f64)[None, :]
    qkvb = qkv_w64 @ np.asarray(ln1_b, f64) + qkv_bias
    fc1_w64 = np.asarray(fc1_w, f64)
    fc1_w_f = fc1_w64 * np.asarray(ln2_w, f64)[None, :]
    fb1 = fc1_w64 @ np.asarray(ln2_b, f64) + np.asarray(fc1_b, f64)

    shared = {
        "qkv_wT": np.ascontiguousarray(qkv_w_f.T).astype(np.float32),
        "proj_wTb": np.ascontiguousarray(np.asarray(proj_w, np.float32).T).astype(ml_dtypes.bfloat16),
        "fc1_wT": np.ascontiguousarray(fc1_w_f.T).astype(np.float32),
        "fc2_wT": np.ascontiguousarray(np.asarray(fc2_w, np.float32).T),
        "qkvb": qkvb.astype(np.float32),
        "pbg1": (np.asarray(gamma1, f64) * np.asarray(proj_b, f64)).astype(np.float32),
        "g1": np.asarray(gamma1, np.float32),
        "fb1": fb1.astype(np.float32),
        "fb2g2": (np.asarray(gamma2, f64) * np.asarray(fc2_b, f64)).astype(np.float32),
        "g2": np.asarray(gamma2, np.float32),
    }
    in_maps = []
    for i in range(NCORES):
        m = dict(shared)
        m["x_fm"] = np.ascontiguousarray(x[i].T)
        in_maps.append(m)

    res = run_bass_kernel_spmd(nc, in_maps, core_ids=list(range(NCORES)))
    out = np.stack([res.results[i]["out_fm"].T for i in range(NCORES)], axis=0)
    return np.ascontiguousarray(out, dtype=np.float32)


# revision 21
# speedup vs baseline: 1.0533x; 1.0533x over previous
"""Transformer block (dense_transformer) on 8 Trainium2 NeuronCores.

Sharding: pure data-parallel — batch B=8, one batch element per core; zero
collectives. Each core runs the full block on its [1024, 768] slice.

On-device layout: the residual stream is kept FEATURE-major ([768, 1024]:
features on SBUF partitions in 6 chunks of 128, tokens on the free axis).
The host transposes x once on the way in and the output once on the way out.

Precision: matmuls in float32r (e8m11, rounded on read, full PE rate at
free-dim >= 256); attention-internal tensors (exp(S), V, attn_out) in bf16;
PSUM accumulation fp32.  LayerNorm affine (w, b) is folded into the
downstream weights on the host.

Attention: scores are computed directly TRANSPOSED (S^T[tk,tq] = K Q^T per
head), exp'd without max-subtraction (scores are O(1); exp is safe in fp32),
and the AV matmul uses V augmented with a ones-column so the softmax
denominator Z falls out as row 64 of the output.  1/Z is applied before proj
via a GpSimd partition broadcast.
"""
import numpy as np
from contextlib import ExitStack

import concourse.bass as bass
import concourse.tile as tile
from concourse import bacc, mybir
from concourse.bass_utils import run_bass_kernel_spmd

f32 = mybir.dt.float32
f32r = mybir.dt.float32r
bf16 = mybir.dt.bfloat16
AF = mybir.ActivationFunctionType
OP = mybir.AluOpType

NCORES = 8
NT = 1024          # tokens per core (batch element)
DIM = 768
CD = DIM // 128    # 6 feature chunks
NH = 12
HD = 64
HID = 3072
HM = HID // 128    # 24 hidden chunks
EPS = 1e-5
HALVES = (slice(0, 512), slice(512, 1024))

_CACHE = {}


def _build(debug=False):
    nc = bacc.Bacc("TRN2", target_bir_lowering=False, debug=False,
                   enable_asserts=False, num_devices=NCORES)
    d = {}
    d["x_fm"] = nc.dram_tensor("x_fm", [DIM, NT], f32r, kind="ExternalInput").ap()
    d["qkv_wT"] = nc.dram_tensor("qkv_wT", [DIM, 3 * DIM], f32r, kind="ExternalInput").ap()
    d["proj_wTb"] = nc.dram_tensor("proj_wTb", [DIM, DIM], bf16, kind="ExternalInput").ap()
    d["fc1_wT"] = nc.dram_tensor("fc1_wT", [DIM, HID], f32r, kind="ExternalInput").ap()
    d["fc2_wT"] = nc.dram_tensor("fc2_wT", [HID, DIM], f32r, kind="ExternalInput").ap()
    d["qkvb"] = nc.dram_tensor("qkvb", [3 * DIM], f32, kind="ExternalInput").ap()
    d["pbg1"] = nc.dram_tensor("pbg1", [DIM], f32, kind="ExternalInput").ap()
    d["g1"] = nc.dram_tensor("g1", [DIM], f32, kind="ExternalInput").ap()
    d["fb1"] = nc.dram_tensor("fb1", [HID], f32, kind="ExternalInput").ap()
    d["fb2g2"] = nc.dram_tensor("fb2g2", [DIM], f32, kind="ExternalInput").ap()
    d["g2"] = nc.dram_tensor("g2", [DIM], f32, kind="ExternalInput").ap()
    out_fm = nc.dram_tensor("out_fm", [DIM, NT], f32, kind="ExternalOutput").ap()
    dbg = {}
    if debug:
        dbg["xn"] = nc.dram_tensor("dbg_xn", [CD, 128, NT], f32, kind="ExternalOutput").ap()
        dbg["mub"] = nc.dram_tensor("dbg_mub", [128, NT], f32, kind="ExternalOutput").ap()
        dbg["rstdb"] = nc.dram_tensor("dbg_rstdb", [128, NT], f32, kind="ExternalOutput").ap()
        dbg["q"] = nc.dram_tensor("dbg_q", [CD, 128, NT], bf16, kind="ExternalOutput").ap()
        dbg["k"] = nc.dram_tensor("dbg_k", [CD, 128, NT], bf16, kind="ExternalOutput").ap()
        dbg["v"] = nc.dram_tensor("dbg_v", [128, 8, NH, HD + 1], bf16, kind="ExternalOutput").ap()
        dbg["e0"] = nc.dram_tensor("dbg_e0", [8, 128, NT], bf16, kind="ExternalOutput").ap()
        dbg["z"] = nc.dram_tensor("dbg_z", [NH, NT], f32, kind="ExternalOutput").ap()
        dbg["rb"] = nc.dram_tensor("dbg_rb", [64, NT], f32, kind="ExternalOutput").ap()
        dbg["ao"] = nc.dram_tensor("dbg_ao", [CD, 128, NT], bf16, kind="ExternalOutput").ap()
        dbg["y1"] = nc.dram_tensor("dbg_y1", [CD, 128, NT], f32, kind="ExternalOutput").ap()
        dbg["h0"] = nc.dram_tensor("dbg_h0", [128, 512], f32, kind="ExternalOutput").ap()

    with tile.TileContext(nc) as tc, ExitStack() as ctx:
        sb = ctx.enter_context(tc.tile_pool(name="sb", bufs=1))

        # ---- constants ----
        ones_f = sb.tile([128, NH, 1], f32, tag="ones_f")
        nc.vector.memset(ones_f, 1.0)
        ones_r = sb.tile([128, 1], f32r, tag="ones_r")
        nc.vector.tensor_copy(out=ones_r, in_=ones_f[:, 0, :])
        ones_b = sb.tile([128, 1], bf16, tag="ones_b")
        nc.vector.tensor_copy(out=ones_b, in_=ones_f[:, 0, :])
        eps_t = sb.tile([128, 1], f32, tag="eps")
        nc.vector.memset(eps_t, EPS)
        qkvb_t = sb.tile([128, 18], f32, tag="qkvb")
        nc.sync.dma_start(out=qkvb_t, in_=d["qkvb"].rearrange("(c p) -> p c", p=128))
        g1_t = sb.tile([128, CD], f32, tag="g1")
        nc.sync.dma_start(out=g1_t, in_=d["g1"].rearrange("(c p) -> p c", p=128))
        pbg1_t = sb.tile([128, CD], f32, tag="pbg1")
        nc.sync.dma_start(out=pbg1_t, in_=d["pbg1"].rearrange("(c p) -> p c", p=128))
        g2_t = sb.tile([128, CD], f32, tag="g2")
        nc.sync.dma_start(out=g2_t, in_=d["g2"].rearrange("(c p) -> p c", p=128))
        fb2g2_t = sb.tile([128, CD], f32, tag="fb2g2")
        nc.sync.dma_start(out=fb2g2_t, in_=d["fb2g2"].rearrange("(c p) -> p c", p=128))
        fb1_t = sb.tile([128, HM], f32, tag="fb1")
        nc.sync.dma_start(out=fb1_t, in_=d["fb1"].rearrange("(c p) -> p c", p=128))
        vb_row = sb.tile([1, DIM], f32, tag="vb_row")
        nc.sync.dma_start(out=vb_row, in_=d["qkvb"][2 * DIM:3 * DIM])
        vb_b = sb.tile([128, DIM], f32, tag="vb_b")
        nc.gpsimd.partition_broadcast(vb_b, vb_row)

        # ---- persistent / streamed activations ----
        x_t = sb.tile([128, CD, NT], f32r, tag="x")          # slot reused for out_sb
        xn_t = sb.tile([128, CD, NT], f32r, tag="xn")        # slot reused for y1n
        v_aug = sb.tile([128, 8, NH, HD + 1], bf16, tag="vaug")
        ao_t = sb.tile([128, CD, NT], bf16, tag="ao")        # attn_out, feature-major
        y1_t = sb.tile([128, CD, NT], f32r, tag="y1")

        for kc in range(CD):
            nc.sync.dma_start(out=x_t[:, kc, :], in_=d["x_fm"][kc * 128:(kc + 1) * 128, :])

        # ================= helpers =================
        # weight loads rotate across DMA queues so no single engine queue
        # serializes the stream
        _dmaq = [nc.sync, nc.gpsimd, nc.scalar]
        _dmaq_i = [0]

        def wdma(out, in_):
            q = _dmaq[_dmaq_i[0] % len(_dmaq)]
            _dmaq_i[0] += 1
            q.dma_start(out=out, in_=in_)

        qkvw_r = d["qkv_wT"].rearrange("(c p) m -> p c m", p=128)
        projw_r = d["proj_wTb"].rearrange("(c p) m -> p c m", p=128)
        fc1w_r = d["fc1_wT"].rearrange("(c p) m -> p c m", p=128)

        def ln_feature_major(sfx, src, dst):
            """dst = (src - mean)/sqrt(var+eps); stats over features (partitions).
            src f32r [128, CD, NT] (read via bitcast for fp32 math); dst f32r."""
            src_f = src.bitcast(f32)
            xsq = {}
            for kc in range(CD):
                for hi, sl in enumerate(HALVES):
                    xq = sb.tile([128, 512], bf16, tag="e", bufs=16,
                                 name=f"xsq{sfx}_{kc}_{hi}")
                    nc.vector.tensor_mul(xq, src_f[:, kc, sl], src_f[:, kc, sl])
                    xsq[(kc, hi)] = xq
            with tc.tile_pool(name=f"ps_ln{sfx}", bufs=1, space="PSUM") as ps:
                s1 = ps.tile([1, NT], f32, tag="s1")
                s2 = ps.tile([1, NT], f32, tag="s2")
                for hi, sl in enumerate(HALVES):
                    for kc in range(CD):
                        nc.tensor.matmul(s1[0:1, sl], ones_r, src[:, kc, sl],
                                         start=(kc == 0), stop=(kc == CD - 1))
                    for kc in range(CD):
                        nc.tensor.matmul(s2[0:1, sl], ones_b, xsq[(kc, hi)],
                                         start=(kc == 0), stop=(kc == CD - 1))
                mu = sb.tile([1, NT], f32, tag="stt", bufs=3, name=f"mu{sfx}")
                nc.vector.tensor_scalar_mul(mu, s1[0:1, :], 1.0 / DIM)
                ex2 = sb.tile([1, NT], f32, tag="stt", bufs=3, name=f"ex2{sfx}")
                nc.vector.tensor_scalar_mul(ex2, s2[0:1, :], 1.0 / DIM)
            mu_b = sb.tile([128, NT], f32, tag="bc", bufs=2, name=f"mub{sfx}")
            nc.gpsimd.partition_broadcast(mu_b, mu)
            # rstd math in a [128, 8] layout — single-partition [1,1024] DVE
            # ops cost ~6.5us each; reshaped they are ~0.1us.
            mu_p = sb.tile([128, 8], f32, tag="stp", bufs=4, name=f"mup{sfx}")
            nc.scalar.dma_start(out=mu_p, in_=mu)
            ex2_p = sb.tile([128, 8], f32, tag="stp", bufs=4, name=f"ex2p{sfx}")
            nc.scalar.dma_start(out=ex2_p, in_=ex2)
            nc.vector.tensor_mul(mu_p, mu_p, mu_p)          # mu^2
            nc.vector.tensor_sub(ex2_p, ex2_p, mu_p)        # var
            nc.scalar.activation(out=ex2_p, in_=ex2_p, func=AF.Sqrt, bias=eps_t)
            nc.vector.reciprocal(ex2_p, ex2_p)              # rstd
            rstd_row = sb.tile([1, NT], f32, tag="stt", bufs=3, name=f"rsr{sfx}")
            nc.scalar.dma_start(out=rstd_row, in_=ex2_p)
            rstd_b = sb.tile([128, NT], f32, tag="bc", bufs=2, name=f"rstdb{sfx}")
            nc.gpsimd.partition_broadcast(rstd_b, rstd_row)
            if debug and sfx == "1":
                nc.gpsimd.dma_start(out=dbg["mub"], in_=mu_b)
                nc.gpsimd.dma_start(out=dbg["rstdb"], in_=rstd_b)
            for kc in range(CD):
                t = sb.tile([128, NT], f32, tag="scr", bufs=2, name=f"lnt{sfx}_{kc}")
                nc.vector.tensor_sub(t, src_f[:, kc, :], mu_b)
                nc.vector.tensor_mul(dst[:, kc, :], t, rstd_b)

        # ================= LN1 =================
        ln_feature_major("1", x_t, xn_t)
        if debug:
            for kc in range(CD):
                nc.gpsimd.dma_start(out=dbg["xn"][kc], in_=xn_t[:, kc, :].bitcast(f32))

        # ====== QKV + attention (shared PSUM pool: 2+2+4 = 8 banks) ======
        q_tiles = {}
        k_tiles = {}
        with tc.tile_pool(name="ps_qa", bufs=1, space="PSUM") as ps:
            # --- v FIRST (attention depends on it; emitting it after q/k
            # deadlocks the mm-tag slot cycle against the qk-tile cycle) ---
            for t in range(8):
                nc.vector.tensor_copy(out=v_aug[:, t, :, HD:HD + 1], in_=ones_f)
            wv_tiles = []
            for kc in range(CD):
                wv = sb.tile([128, DIM], f32r, tag="w768", bufs=6, name=f"wv{kc}")
                nc.sync.dma_start(out=wv, in_=d["qkv_wT"][kc * 128:(kc + 1) * 128,
                                                          2 * DIM:3 * DIM])
                wv_tiles.append(wv)
            for t in range(8):
                for j in range(2):
                    vsl = slice(j * 384, (j + 1) * 384)
                    acc = ps.tile([128, 384], f32, tag="mm", bufs=2, name=f"v_ps{t}_{j}")
                    for kc in range(CD):
                        nc.tensor.matmul(acc,
                                         xn_t[:, kc, t * 128:(t + 1) * 128],
                                         wv_tiles[kc][:, vsl],
                                         start=(kc == 0), stop=(kc == CD - 1))
                    nc.vector.tensor_add(
                        v_aug[:, t, 6 * j:6 * (j + 1), 0:HD],
                        acc.rearrange("p (h d) -> p h d", h=6),
                        vb_b[:, vsl].rearrange("p (h d) -> p h d", h=6))

            if debug:
                nc.gpsimd.dma_start(out=dbg["v"], in_=v_aug)

            # --- q (m 0..5) / k (m 6..11), interleaved per pair ---
            for p in range(CD):
                for m in (p, p + CD):
                    w6 = sb.tile([128, CD, 128], f32r, tag="w6", bufs=4,
                                 name=f"wqk{m}")
                    wdma(w6, qkvw_r[:, :, m * 128:(m + 1) * 128])
                    dst = sb.tile([128, NT], bf16, tag="qk", bufs=4,
                                  name=f"{'q' if m < CD else 'k'}{p}")
                    for sl in HALVES:
                        acc = ps.tile([128, 512], f32, tag="mm", bufs=2, name=f"qk_ps{m}")
                        for kc in range(CD):
                            nc.tensor.matmul(acc, w6[:, kc, :], xn_t[:, kc, sl],
                                             start=(kc == 0), stop=(kc == CD - 1))
                        if m < CD:  # q: (psum + qb) * head_dim^-0.5
                            nc.vector.tensor_scalar(out=dst[:, sl], in0=acc,
                                                    scalar1=qkvb_t[:, m:m + 1],
                                                    scalar2=float(HD) ** -0.5,
                                                    op0=OP.add, op1=OP.mult)
                        else:       # k: psum + kb
                            nc.scalar.activation(out=dst[:, sl], in_=acc,
                                                 func=AF.Identity,
                                                 bias=qkvb_t[:, m:m + 1])
                    if m < CD:
                        q_tiles[p] = dst
                        if debug:
                            nc.gpsimd.dma_start(out=dbg["q"][p], in_=dst)
                    else:
                        k_tiles[p] = dst
                        if debug:
                            nc.gpsimd.dma_start(out=dbg["k"][p], in_=dst)

            # --- attention, head pair p -> heads (2p, 2p+1) ---
            for p in range(CD):
                avps = [ps.tile([128, NT], f32, tag="av", bufs=2, name=f"av{p}_{h2}")
                        for h2 in range(2)]
                for s in range(8):
                    for hi, sl in enumerate(HALVES):
                        # the two heads' S^T matmuls issue back-to-back into
                        # different row groups -> they run concurrently on PE
                        stps = []
                        for h2 in range(2):
                            base = h2 * 64
                            stp = ps.tile([128, 512], f32, tag="st", bufs=2,
                                          name=f"st{p}_{s}_{h2}_{hi}")
                            nc.tensor.matmul(stp,
                                             k_tiles[p][base:base + 64,
                                                        s * 128:(s + 1) * 128],
                                             q_tiles[p][base:base + 64, sl],
                                             start=True, stop=True,
                                             tile_position=(base, 0))
                            stps.append(stp)
                        for h2 in range(2):
                            eT = sb.tile([128, 512], bf16, tag="e", bufs=16,
                                         name=f"e{p}_{s}_{h2}_{hi}")
                            nc.scalar.activation(out=eT, in_=stps[h2], func=AF.Exp)
                            if debug and p == 0 and h2 == 0:
                                nc.gpsimd.dma_start(out=dbg["e0"][s, :, sl], in_=eT)
                            nc.tensor.matmul(avps[h2][0:HD + 1, sl],
                                             v_aug[:, s, 2 * p + h2, :],
                                             eT,
                                             start=(s == 0), stop=(s == 7))
                for h2 in range(2):
                    # Z row lives at PSUM partition 64.  Copy it out (ACT),
                    # reshape to [128, 8] via DMA so the reciprocal runs on
                    # 128 lanes, reshape back to partition 0, broadcast.
                    zs = sb.tile([65, NT], f32, tag="scr", bufs=2, name=f"zs{p}_{h2}")
                    nc.scalar.copy(out=zs[HD:HD + 1, :], in_=avps[h2][HD:HD + 1, :])
                    zp = sb.tile([128, 8], f32, tag="stp", bufs=4, name=f"zp{p}_{h2}")
                    nc.scalar.dma_start(out=zp, in_=zs[HD:HD + 1, :])
                    nc.vector.reciprocal(zp, zp)
                    zr = sb.tile([1, NT], f32, tag="stt", bufs=3, name=f"zr{p}_{h2}")
                    nc.scalar.dma_start(out=zr, in_=zp)
                    if debug:
                        nc.gpsimd.dma_start(out=dbg["z"][2 * p + h2], in_=zr)
                    r_b = sb.tile([64, NT], f32, tag="bc", bufs=2, name=f"rb{p}_{h2}")
                    nc.gpsimd.partition_broadcast(r_b, zr)
                    if debug and p == 0 and h2 == 0:
                        nc.gpsimd.dma_start(out=dbg["rb"], in_=r_b)
                    if h2 == 0:
                        nc.vector.tensor_mul(ao_t[0:64, p, :], avps[0][0:64, :], r_b)
                    else:
                        tmp = sb.tile([64, NT], bf16, tag="h", bufs=2, name=f"sh{p}")
                        nc.vector.tensor_mul(tmp, avps[1][0:64, :], r_b)
                        nc.gpsimd.dma_start(out=ao_t[64:128, p, :], in_=tmp)

        # ================= proj + residual =================
        with tc.tile_pool(name="ps_proj", bufs=1, space="PSUM") as ps:
            for m in range(CD):
                w6 = sb.tile([128, CD, 128], bf16, tag="w6", bufs=4, name=f"wpj{m}")
                wdma(w6, projw_r[:, :, m * 128:(m + 1) * 128])
                for sl in HALVES:
                    acc = ps.tile([128, 512], f32, tag="mm", bufs=2, name=f"pj_ps{m}")
                    for cc in range(CD):
                        nc.tensor.matmul(acc, w6[:, cc, :], ao_t[:, cc, sl],
                                         start=(cc == 0), stop=(cc == CD - 1))
                    tmp = sb.tile([128, 512], f32, tag="scr512", bufs=2, name=f"pjt{m}")
                    nc.scalar.activation(out=tmp, in_=acc, func=AF.Identity,
                                         bias=pbg1_t[:, m:m + 1], scale=g1_t[:, m:m + 1])
                    nc.vector.tensor_add(y1_t[:, m, sl], tmp, x_t[:, m, sl].bitcast(f32))

        if debug:
            for kc in range(CD):
                nc.gpsimd.dma_start(out=dbg["ao"][kc], in_=ao_t[:, kc, :])
                nc.gpsimd.dma_start(out=dbg["y1"][kc], in_=y1_t[:, kc, :].bitcast(f32))

        # ================= LN2 =================
        y1n_t = sb.tile([128, CD, NT], f32r, tag="xn", name="y1n")
        ln_feature_major("2", y1_t, y1n_t)

        # ================= MLP =================
        out_sb = sb.tile([128, CD, NT], f32, tag="x", name="out_sb")
        with tc.tile_pool(name="ps_mlp", bufs=1, space="PSUM") as ps:
            for hi, sl in enumerate(HALVES):
                acc2 = ps.tile([128, CD, 512], f32, tag="fc2", bufs=1, name=f"fc2_ps{hi}")
                for hm in range(HM):
                    fps = ps.tile([128, 512], f32, tag="fc1", bufs=2,
                                  name=f"fc1_ps{hi}_{hm}")
                    w6 = sb.tile([128, CD, 128], f32r, tag="w6", bufs=4,
                                 name=f"wf1_{hi}_{hm}")
                    wdma(w6, fc1w_r[:, :, hm * 128:(hm + 1) * 128])
                    for cc in range(CD):
                        nc.tensor.matmul(fps, w6[:, cc, :], y1n_t[:, cc, sl],
                                         start=(cc == 0), stop=(cc == CD - 1))
                    h_t = sb.tile([128, 512], f32r, tag="h", bufs=2, name=f"h{hi}_{hm}")
                    nc.scalar.activation(out=h_t, in_=fps, func=AF.Gelu,
                                         bias=fb1_t[:, hm:hm + 1])
                    if debug and hi == 0 and hm == 0:
                        nc.gpsimd.dma_start(out=dbg["h0"], in_=h_t.bitcast(f32))
                    w2 = sb.tile([128, DIM], f32r, tag="w768", bufs=6,
                                 name=f"wf2_{hi}_{hm}")
                    wdma(w2, d["fc2_wT"][hm * 128:(hm + 1) * 128, :])
                    for m in range(CD):
                        nc.tensor.matmul(acc2[:, m, :], w2[:, m * 128:(m + 1) * 128], h_t,
                                         start=(hm == 0), stop=(hm == HM - 1))
                for m in range(CD):
                    tmp = sb.tile([128, 512], f32, tag="scr512", bufs=2,
                                  name=f"of{hi}_{m}")
                    nc.scalar.activation(out=tmp, in_=acc2[:, m, :], func=AF.Identity,
                                         bias=fb2g2_t[:, m:m + 1], scale=g2_t[:, m:m + 1])
                    nc.vector.tensor_add(out_sb[:, m, sl], tmp,
                                         y1_t[:, m, sl].bitcast(f32))

        for kc in range(CD):
            nc.gpsimd.dma_start(out=out_fm[kc * 128:(kc + 1) * 128, :],
                                in_=out_sb[:, kc, :])

    nc.compile()
    return nc


def kernel(x, ln1_w, ln1_b, qkv_w, q_bias, v_bias, proj_w, proj_b,
           ln2_w, ln2_b, fc1_w, fc1_b, fc2_w, fc2_b, gamma1, gamma2):
    import ml_dtypes
    if "nc" not in _CACHE:
        _CACHE["nc"] = _build()
    nc = _CACHE["nc"]

    f64 = np.float64
    x = np.asarray(x, np.float32)
    # Fold LN affine params into downstream weights (exact, on host):
    #   qkv(ln1(x)) = (qkv_w * ln1_w) @ xhat + (qkv_w @ ln1_b + qkv_bias)
    qkv_w64 = np.asarray(qkv_w, f64)
    qkv_bias = np.concatenate([np.asarray(q_bias, f64),
                               np.zeros(DIM, f64),
                               np.asarray(v_bias, f64)])
    qkv_w_f = qkv_w64 * np.asarray(ln1_w, f64)[None, :]
    qkvb = qkv_w64 @ np.asarray(ln1_b, f64) + qkv_bias
    fc1_w64 = np.asarray(fc1_w, f64)
    fc1_w_f = fc1_w64 * np.asarray(ln2_w, f64)[None, :]
    fb1 = fc1_w64 @ np.asarray(ln2_b, f64) + np.asarray(fc1_b, f64)

    shared = {
        "qkv_wT": np.ascontiguousarray(qkv_w_f.T).astype(np.float32),
        "proj_wTb": np.ascontiguousarray(np.asarray(proj_w, np.float32).T).astype(ml_dtypes.bfloat16),
        "fc1_wT": np.ascontiguousarray(fc1_w_f.T).astype(np.float32),
        "fc2_wT": np.ascontiguousarray(np.asarray(fc2_w, np.float32).T),
        "qkvb": qkvb.astype(np.float32),
        "pbg1": (np.asarray(gamma1, f64) * np.asarray(proj_b, f64)).astype(np.float32),
        "g1": np.asarray(gamma1, np.float32),
        "fb1": fb1.astype(np.float32),
        "fb2g2": (np.asarray(gamma2, f64) * np.asarray(fc2_b, f64)).astype(np.float32),
        "g2": np.asarray(gamma2, np.float32),
    }
    in_maps = []
    for i in range(NCORES):
        m = dict(shared)
        m["x_fm"] = np.ascontiguousarray(x[i].T)
        in_maps.append(m)

    res = run_bass_kernel_spmd(nc, in_maps, core_ids=list(range(NCORES)))
    out = np.stack([res.results[i]["out_fm"].T for i in range(NCORES)], axis=0)
    return np.ascontiguousarray(out, dtype=np.float32)
